# revision 9
# baseline (speedup 1.0000x reference)
"""Trainium2 Bass kernel for the ConvMod problem:

    Y1 = valid 2x2 cross-correlation(X, W)    # [4095, 4095]
    Y2 = transposed-conv(Y1, W)               # [4096, 4096]

The composite equals, in the interior, a 3x3 convolution of X with
K = corr(W, W), plus boundary corrections coming from the clipping of
Y1's domain:

    Y2 = Conv3x3_zeropad(X; K) - E_row - E_col + E_both

  E_row[0, q]    = sum_{b,d} W[1,b] W[1,d] Xpad[0,    q+b-d]
  E_row[H-1, q]  = sum_{b,d} W[0,b] W[0,d] Xpad[H-1,  q+b-d]
  E_col[p, 0]    = sum_{a,c} W[a,1] W[c,1] Xpad[p+a-c, 0]
  E_col[p, L-1]  = sum_{a,c} W[a,0] W[c,0] Xpad[p+a-c, L-1]
  E_both (corners): (0,0): W[1,1]^2 X[0,0]; (0,L-1): W[1,0]^2 X[0,L-1];
                    (H-1,0): W[0,1]^2 X[H-1,0]; (H-1,L-1): W[0,0]^2 X[...].

Distribution: data-parallel over rows across 8 cores; each core gets a
[514, 4096] row slab of X with a 1-row halo on each side (zero-padded at
the global edges), plus per-core stationary band matrices, and produces
its [512, 4096] slice of Y2.  No collectives.

On-device: rows live on SBUF partitions, columns on the free axis.  For a
block of M output rows we load an X tile of Kin = M+2 rows and run, per
512-column chunk, three TensorE matmuls with tridiagonal stationary band
matrices (one per column offset v in {-1,0,+1}; the column shift is
applied on the moving operand's free-axis slice), all accumulating into
one PSUM bank.  N=1 correction matmuls fix output columns 0 and L-1.
Row-boundary corrections are baked into the per-core stationary matrices.
PSUM is evacuated to SBUF alternately on ScalarE/VectorE and DMA'd out.
Matmuls run as float32r (input data is fp32 bit-identical; the PE runs
its fast reduced-precision fp32 path; accumulation is fp32).
"""

import numpy as np

import concourse.bass as bass
from concourse import bacc
import concourse.mybir as mybir
from concourse.tile import TileContext
from concourse.bass_utils import run_bass_kernel_spmd

H = 4096
L = 4096
LEXT = 4096 + 6            # slab columns: X row + [X0, 0, X0, XL, 0, XL] staging
NCORES = 8
RPC = H // NCORES          # output rows per core: 512
SLAB = RPC + 2             # input slab rows per core (1-row halo each side)
BLOCK_MS = [126, 126, 126, 126, 8]
BLOCK_STARTS = [0, 126, 252, 378, 504]
CHUNK = 512
NCH = L // CHUNK
WPAD_K = 128
WPAD_M = 126
NMATS = 15                 # 3 sets x 5 matrices
F32 = mybir.dt.float32
F32R = mybir.dt.float32r


# ----------------------------------------------------------------------------
# Host-side stationary-matrix construction
# ----------------------------------------------------------------------------

def _make_taps(W):
    W = np.asarray(W, dtype=np.float64)
    K = np.zeros((3, 3))
    for a in range(2):
        for b in range(2):
            for c in range(2):
                for d in range(2):
                    K[a - c + 1, b - d + 1] += W[a, b] * W[c, d]
    rowtop = np.zeros(3)
    rowbot = np.zeros(3)
    for b in range(2):
        for d in range(2):
            rowtop[b - d + 1] += W[1, b] * W[1, d]
            rowbot[b - d + 1] += W[0, b] * W[0, d]
    col0 = np.zeros(3)
    colL = np.zeros(3)
    for a in range(2):
        for c in range(2):
            col0[a - c + 1] += W[a, 1] * W[c, 1]
            colL[a - c + 1] += W[a, 0] * W[c, 0]
    corners = {
        (0, 0): W[1, 1] ** 2,
        (0, 1): W[1, 0] ** 2,
        (1, 0): W[0, 1] ** 2,
        (1, 1): W[0, 0] ** 2,
    }
    return K, rowtop, rowbot, col0, colL, corners


def _build_block_mats(W, M, first_row_global, last_row_global):
    """[5, M+2, M]: bands for v=-1,0,+1 then negated C0, C_L corrections."""
    K3, rowtop, rowbot, col0, colL, corners = _make_taps(W)
    Kin = M + 2
    mats = np.zeros((5, Kin, M))
    for m in range(M):
        for u in (-1, 0, 1):
            k = m + 1 + u
            for vi, v in enumerate((-1, 0, 1)):
                mats[vi, k, m] = K3[u + 1, v + 1]
            mats[3, k, m] = -col0[u + 1]
            mats[4, k, m] = -colL[u + 1]
    if first_row_global:
        for vi, v in enumerate((-1, 0, 1)):
            mats[vi, 1, 0] -= rowtop[v + 1]
        mats[3, 1, 0] += corners[(0, 0)]
        mats[4, 1, 0] += corners[(0, 1)]
    if last_row_global:
        m = M - 1
        for vi, v in enumerate((-1, 0, 1)):
            mats[vi, m + 1, m] -= rowbot[v + 1]
        mats[3, m + 1, m] += corners[(1, 0)]
        mats[4, m + 1, m] += corners[(1, 1)]
    return mats


def _build_wstack(W, core):
    """Per-core stationary stack [128, 15*126] (k-major, clean 2D DMA)."""
    out = np.zeros((WPAD_K, 3, 5, WPAD_M), dtype=np.float32)
    b0 = _build_block_mats(W, 126, core == 0, False)
    mid = _build_block_mats(W, 126, False, False)
    b4 = _build_block_mats(W, 8, False, core == NCORES - 1)
    for w in range(5):
        out[:128, 0, w, :126] = b0[w].astype(np.float32)
        out[:128, 1, w, :126] = mid[w].astype(np.float32)
        out[:10, 2, w, :8] = b4[w].astype(np.float32)
    return out.reshape(WPAD_K, NMATS * WPAD_M)


def _make_slabs(X):
    X = np.ascontiguousarray(np.asarray(X, dtype=np.float32))
    slabs = np.zeros((NCORES, SLAB, LEXT), dtype=np.float32)
    for c in range(NCORES):
        lo = c * RPC - 1
        hi = c * RPC + RPC + 1
        src_lo = max(0, lo)
        src_hi = min(H, hi)
        slabs[c, src_lo - lo : src_hi - lo, :L] = X[src_lo:src_hi, :]
    # staging columns for the N=2 edge-fix matmuls (PSUM writes must be
    # 8B-aligned with even N, so single-column terms are expressed as
    # [col, 0] / [0, col] pairs)
    slabs[:, :, L + 0] = slabs[:, :, 0]
    slabs[:, :, L + 2] = slabs[:, :, 0]
    slabs[:, :, L + 3] = slabs[:, :, L - 1]
    slabs[:, :, L + 5] = slabs[:, :, L - 1]
    return slabs


# ----------------------------------------------------------------------------
# Device program (SPMD; identical instruction stream on all 8 cores)
# ----------------------------------------------------------------------------

def build_nc(compile=True):
    nc = bacc.Bacc()
    x_d = nc.declare_dram_parameter("xslab", [SLAB, LEXT], F32R, isOutput=False)
    w_d = nc.declare_dram_parameter("wstack", [WPAD_K, NMATS * WPAD_M], F32R, isOutput=False)
    y_d = nc.declare_dram_parameter("y", [RPC, L], F32, isOutput=True)

    with TileContext(nc) as tc:
        with (
            tc.tile_pool(name="wp", bufs=1) as wp,
            tc.tile_pool(name="xp", bufs=5) as xp,
            tc.tile_pool(name="yp", bufs=4) as yp,
            tc.tile_pool(name="pp", bufs=8, space="PSUM") as pp,
        ):
            wsb = wp.tile([WPAD_K, NMATS * WPAD_M], F32R, name="wsb")
            # set 0 first (the only stationary set the first block needs);
            # sets 1/2 are issued after block 0's load pieces below
            nc.scalar.dma_start(
                out=wsb[:, 0 : 5 * WPAD_M], in_=w_d[:, 0 : 5 * WPAD_M]
            )

            for b in (0, 1, 4, 2, 3):
                M, s = BLOCK_MS[b], BLOCK_STARTS[b]
                Kin = M + 2
                si = 0 if b == 0 else (1 if b < 4 else 2)

                xt = xp.tile([128, LEXT], F32R, name=f"xt{b}", tag="xt")
                # column-split pieces so chunk-0 matmuls start after ~1/4 of
                # the tile has landed; the tiny staging-column piece goes
                # first (the chunk-0 edge matmuls read it)
                nc.scalar.dma_start(
                    out=xt[:Kin, L:LEXT], in_=x_d[s : s + Kin, L:LEXT]
                )
                for pi, (a, b_) in enumerate(
                    ((0, 1024), (1024, 2048), (2048, 3072), (3072, L))
                ):
                    dma_eng = nc.sync if pi % 2 == 0 else nc.scalar
                    dma_eng.dma_start(
                        out=xt[:Kin, a:b_], in_=x_d[s : s + Kin, a:b_]
                    )
                if b == 0:
                    # remaining stationary sets, needed from block 2 onward
                    nc.sync.dma_start(
                        out=wsb[:, 5 * WPAD_M :], in_=w_d[:, 5 * WPAD_M :]
                    )
                yt = yp.tile([128, L], F32, name=f"yt{b}", tag="yt")

                def wm(wi):
                    base = (si * 5 + wi) * WPAD_M
                    return wsb[0:Kin, base : base + M]

                def xr(c0, n):
                    return xt[0:Kin, c0 : c0 + n]

                pts = [
                    pp.tile([128, CHUNK], F32, name=f"pt{b}_{q}", tag="pt")
                    for q in range(NCH)
                ]

                # band v=0: full-width first touch per bank (start=True
                # clears the bank; partial-width bands then accumulate)
                for q in range(NCH):
                    nc.tensor.matmul(
                        pts[q][0:M, 0:CHUNK], wm(1), xr(q * CHUNK, CHUNK),
                        start=True, stop=False,
                    )
                # band v=-1 (psum writes must be 8B-aligned, even N:
                # chunk 0 covers [2:512); cols 0-1 are fixed below)
                nc.tensor.matmul(
                    pts[0][0:M, 2:CHUNK], wm(0), xr(1, CHUNK - 2),
                    start=False, stop=False,
                )
                for q in range(1, NCH):
                    nc.tensor.matmul(
                        pts[q][0:M, 0:CHUNK], wm(0), xr(q * CHUNK - 1, CHUNK),
                        start=False, stop=False,
                    )
                # band v=+1; chunk 0 first so its evacuation starts early
                nc.tensor.matmul(
                    pts[0][0:M, 0:CHUNK], wm(2), xr(1, CHUNK),
                    start=False, stop=False,
                )
                # left-edge fixes via N=2 matmuls on the [X0, 0, X0] staging
                # cols: col 0 += C0 . X0 ; col 1 += Band_-1 . X0
                nc.tensor.matmul(
                    pts[0][0:M, 0:2], wm(3), xr(L, 2), start=False, stop=False
                )
                nc.tensor.matmul(
                    pts[0][0:M, 0:2], wm(0), xr(L + 1, 2), start=False, stop=True
                )
                for q in range(1, NCH - 1):
                    nc.tensor.matmul(
                        pts[q][0:M, 0:CHUNK], wm(2), xr(q * CHUNK + 1, CHUNK),
                        start=False, stop=True,
                    )
                nc.tensor.matmul(
                    pts[NCH - 1][0:M, 0 : CHUNK - 2],
                    wm(2), xr((NCH - 1) * CHUNK + 1, CHUNK - 2),
                    start=False, stop=False,
                )
                # right-edge fixes on [XL, 0, XL]: col 510 += Band_+1 . XL ;
                # col 511 += C_L . XL
                nc.tensor.matmul(
                    pts[NCH - 1][0:M, CHUNK - 2 : CHUNK], wm(2), xr(L + 3, 2),
                    start=False, stop=False,
                )
                nc.tensor.matmul(
                    pts[NCH - 1][0:M, CHUNK - 2 : CHUNK], wm(4), xr(L + 4, 2),
                    start=False, stop=True,
                )

                # evacuate PSUM -> SBUF on two engines, then DMA out
                for q in range(NCH):
                    src = pts[q][0:M, 0:CHUNK]
                    dst = yt[0:M, q * CHUNK : (q + 1) * CHUNK]
                    if q in (0, 4, 6):
                        nc.scalar.copy(dst, src)
                    else:
                        nc.vector.tensor_copy(dst, src)
                # stores on SWDGE (GpSimd issue path is idle), in pieces
                # so they drain while later chunks still compute
                for a in range(0, L, 512):
                    nc.gpsimd.dma_start(
                        out=y_d[s : s + M, a : a + 512],
                        in_=yt[0:M, a : a + 512],
                    )
    if compile:
        nc.compile()
    return nc


_NC_CACHE = None


def _get_nc():
    global _NC_CACHE
    if _NC_CACHE is None:
        _NC_CACHE = build_nc()
    return _NC_CACHE


def _run(X, W, trace=False, **spmd_kwargs):
    slabs = _make_slabs(X)
    in_maps = []
    for c in range(NCORES):
        in_maps.append(
            {"xslab": slabs[c], "wstack": _build_wstack(W, c)}
        )
    res = run_bass_kernel_spmd(
        _get_nc(), in_maps, core_ids=list(range(NCORES)), trace=trace, **spmd_kwargs
    )
    Y = np.concatenate([res.results[c]["y"] for c in range(NCORES)], axis=0)
    return Y, res


def kernel(X, W):
    Y, _ = _run(X, W)
    return Y
